# revision 20
# baseline (speedup 1.0000x reference)
"""GCN (7-layer, PyG GCNConv-style) on 8 Trainium2 NeuronCores.

v3 design (gather-throughput + pipeline optimized):
- Nodes destination-sharded: core k owns nodes [k*12500, (k+1)*12500).
  Within a core, nodes are placed on a [128 partitions x 98 blocks] grid in
  DESCENDING IN-DEGREE order (rank r -> (p=r%128, c=r//128)), so each block's
  128 nodes have nearly equal degree and the per-block slot count
  D_b = max in-block degree wastes <5% slots.
- Per layer: stage A computes htil = dinv*(H @ W) per block (TensorE), casts
  to fp16 and AllGathers a [100352, fo_pad] fp16 table (rows in placement
  order, fo padded so that groups of G=4 rows form a 256B/512B element).
- Aggregation: per destination slot (p, slot j) one int16 GROUP index
  (= src_row//4, < 25088 so it fits dma_gather's int16 limit). Gathers run as
  896-index dma_gather instructions round-robined over 4 SWDGE queues.
  The gathered [128, 7, G*fo] fp16 messages are scaled in-place by host-built
  masked weights wqg[p, slot, g] = w_e * dinv_dst * (g == src%4), then
  block-pieces are reduced over (slot, g) on the VectorE into fp32 accums.
- Self loops fold algebraically: out = relu(acc + dinv^2*(H@W) + b).
- Layer 7 commutes W7 past the aggregation (out = (A_hat h6) @ W7 + b7) so
  its table is dinv*h6 (10 wide) instead of a 1-wide table.
- SOFTWARE PIPELINING: each engine executes its stream in order, so the
  epilogue of layer l, stage A of layer l+1 and the bounce-DMA pieces are
  EMITTED inside layer l's gather-chunk loop at the points where their block
  group's accumulator columns are fully reduced. Only the AllGather and the
  last group's tail remain exposed between layers' gather phases.
- dinv/deg are computed on the host (pure function of edge inputs) and folded
  into wqg; dinv/dinv^2 ship as small [128, 98] parameters for the stage-A
  scaling and self-loop terms.
"""
import sys

sys.path.insert(0, "/opt/trn_rl_repo")

from contextlib import ExitStack

import numpy as np

NC = 8
N_NODES = 100000
NLOC = N_NODES // NC            # 12500
NBLK = (NLOC + 127) // 128      # 98
NLOCP = NBLK * 128              # 12544
NTAB = NC * NLOCP               # 100352 table rows
G = 4                           # nodes per gather element
NGRP = NTAB // G                # 25088 (< 32768, int16-safe)
DIMS = [(128, 50), (50, 50), (50, 30), (30, 30), (30, 10), (10, 10), (10, 1)]
NLAYER = len(DIMS)
# gather-table padded widths per layer (G * FOP * 2B must be % 256)
FOP = [64, 64, 32, 32, 32, 32, 32]   # layer-7 table holds dinv*h6 (10 wide)
FOT = [50, 50, 30, 30, 10, 10, 10]   # real table width per layer
CHUNK_COLS = 7                  # 896 idxs per dma_gather (ucode inflight cap)
DMA_SCRATCH = 16384
EPB = 14                        # blocks per epilogue/stage-A group
HALF_BLK = 4 * EPB              # blocks in table half A (AllGather split)


def _build_program(NSLOT, colbase):
    from concourse import bass, bacc, mybir, tile

    NCHUNK = NSLOT // CHUNK_COLS
    groups = [(g0, min(g0 + EPB, NBLK)) for g0 in range(0, NBLK, EPB)]

    def ready_ci(g1):
        # chunk index after which all slot columns < colbase[g1] are reduced
        return (int(colbase[g1]) + CHUNK_COLS - 1) // CHUNK_COLS - 1

    nc = bacc.Bacc(
        "TRN2",
        target_bir_lowering=False,
        debug=False,
        num_devices=NC,
        num_swdge_queues=4,
        dynamic_dma_scratch_size=DMA_SCRATCH,
    )

    f16, f32 = mybir.dt.float16, mybir.dt.float32
    bf16 = mybir.dt.bfloat16
    x_p = nc.declare_dram_parameter("x_p", [128, NBLK * 128], bf16, isOutput=False)
    idx_w = nc.declare_dram_parameter("idx_w", [128, NSLOT * 8], mybir.dt.int16, isOutput=False)
    wqg_d = nc.declare_dram_parameter("wqg", [128, NSLOT * G], f16, isOutput=False)
    ident = nc.declare_dram_parameter("ident", [128, 128], f32, isOutput=False)
    dinv_d = nc.declare_dram_parameter("dinv_p", [128, NBLK], f32, isOutput=False)
    dinv2_d = nc.declare_dram_parameter("dinv2_p", [128, NBLK], f32, isOutput=False)
    Ws, Bs = [], []
    for i, (fi, fo) in enumerate(DIMS):
        Ws.append(nc.declare_dram_parameter(f"W{i+1}", [fi, fo], f32, isOutput=False))
        Bs.append(nc.declare_dram_parameter(f"b{i+1}", [128, fo], f32, isOutput=False))
    w7row = nc.declare_dram_parameter("w7row", [128, 10], f32, isOutput=False)
    out_ext = nc.declare_dram_parameter("out", [128, NBLK], f32, isOutput=True)

    f8 = mybir.dt.float8e4
    tdt = [f8, f8] + [f16] * (NLAYER - 2)   # table dtype per layer
    bounces = [nc.dram_tensor(f"bounce{i}", [NLOCP, FOP[i]], tdt[i]) for i in range(NLAYER)]
    tables = [
        nc.dram_tensor(f"table{i}", [NTAB, FOP[i]], tdt[i], addr_space="Shared")
        for i in range(NLAYER)
    ]

    with tile.TileContext(nc) as tc, ExitStack() as ctx:
        const = ctx.enter_context(tc.tile_pool(name="const", bufs=1))
        work = ctx.enter_context(tc.tile_pool(name="work", bufs=4))
        hpool = ctx.enter_context(tc.tile_pool(name="hpool", bufs=2))
        tpool = ctx.enter_context(tc.tile_pool(name="tpool", bufs=2))
        t2pool = ctx.enter_context(tc.tile_pool(name="t2pool", bufs=2))
        msgp = ctx.enter_context(tc.tile_pool(name="msgp", bufs=5))
        accp = ctx.enter_context(tc.tile_pool(name="accp", bufs=2))
        psT = ctx.enter_context(tc.tile_pool(name="psT", bufs=4, space="PSUM"))
        psH = ctx.enter_context(tc.tile_pool(name="psH", bufs=4, space="PSUM"))

        ident_t = const.tile([128, 128], f32)
        nc.sync.dma_start(out=ident_t[:], in_=ident[:])
        ident_b = const.tile([128, 128], bf16)
        nc.vector.tensor_copy(out=ident_b[:], in_=ident_t[:])
        w7row_t = const.tile([128, 10], f32)
        nc.sync.dma_start(out=w7row_t[:], in_=w7row[:])
        idx_t = const.tile([128, NSLOT * 8], mybir.dt.int16)
        nc.sync.dma_start(out=idx_t[:], in_=idx_w[:])
        wqg_t = const.tile([128, NSLOT, G], f16)
        nc.sync.dma_start(out=wqg_t[:], in_=wqg_d[:].rearrange("p (s g) -> p s g", g=G))
        dinv_t = const.tile([128, NBLK], f32)
        nc.sync.dma_start(out=dinv_t[:], in_=dinv_d[:])
        dinv2_t = const.tile([128, NBLK], f32)
        nc.sync.dma_start(out=dinv2_t[:], in_=dinv2_d[:])
        W_ts, B_ts, Wb_ts = [], [], []
        for i, (fi, fo) in enumerate(DIMS):
            W_t = const.tile([fi, fo], f32, tag=f"W{i}")
            nc.sync.dma_start(out=W_t[:], in_=Ws[i][:])
            W_b = const.tile([fi, fo], bf16, tag=f"Wb{i}")
            nc.vector.tensor_copy(out=W_b[:], in_=W_t[:])
            B_t = const.tile([128, fo], f32, tag=f"B{i}")
            nc.sync.dma_start(out=B_t[:], in_=Bs[i][:])
            W_ts.append(W_t)
            B_ts.append(B_t)
            Wb_ts.append(W_b)

        nidx_reg = nc.gpsimd.to_reg(CHUNK_COLS * 128)

        htil_t = [None] * NLAYER
        htil2_t = [None] * NLAYER
        h_t = [None] * NLAYER

        def alloc_layer_tiles(li):
            fot, fop = FOT[li], FOP[li]
            fo = DIMS[li][1]
            htil_t[li] = tpool.tile([128, NBLK, fop], tdt[li], tag="htil", name=f"htil{li}")
            if fot < fop:
                nc.vector.memset(htil_t[li][:], 0.0)
            if li < NLAYER - 1:
                htil2_t[li] = t2pool.tile([128, NBLK, fo], f32, tag="htil2", name=f"htil2_{li}")

        def emit_stageA_block(li, b, src_ap):
            # src_ap: [128, fi] bf16 rows of the layer's input h
            fi, fo = DIMS[li]
            fot = FOT[li]
            if li == NLAYER - 1:
                # commuted layer 7: table = dinv * h6
                nc.scalar.activation(
                    out=htil_t[li][:, b, :fot],
                    in_=src_ap,
                    func=mybir.ActivationFunctionType.Copy,
                    scale=dinv_t[:, b : b + 1],
                )
                return
            pT = psT.tile([fi, 128], bf16, space="PSUM", tag="pT")
            nc.tensor.transpose(out=pT[:], in_=src_ap, identity=ident_b[:])
            hT = work.tile([fi, 128], bf16, tag="hT")
            nc.scalar.copy(out=hT[:], in_=pT[:])
            pH = psH.tile([128, fo], f32, space="PSUM", tag="pH")
            nc.tensor.matmul(out=pH[:], lhsT=hT[:], rhs=Wb_ts[li][:], start=True, stop=True)
            nc.scalar.activation(
                out=htil_t[li][:, b, :fot],
                in_=pH[:],
                func=mybir.ActivationFunctionType.Copy,
                scale=dinv_t[:, b : b + 1],
            )
            nc.vector.scalar_tensor_tensor(
                out=htil2_t[li][:, b, :],
                in0=pH[:],
                scalar=dinv2_t[:, b : b + 1],
                in1=B_ts[li][:],
                op0=mybir.AluOpType.mult,
                op1=mybir.AluOpType.add,
            )

        def emit_bounce_piece(li, g0, g1):
            nc.sync.dma_start(
                out=bounces[li][:].rearrange("(c p) f -> p c f", p=128)[:, g0:g1, :],
                in_=htil_t[li][:, g0:g1, :],
            )

        HALF_G = 4                      # groups 0..3 -> blocks 0..56
        ROWS_A = groups[HALF_G - 1][1] * 128   # 7168 bounce rows in half A

        def emit_allgather(li, half):
            # table rows: [all cores' half-A rows][all cores' half-B rows],
            # so each half's AllGather output is contiguous
            r0, r1 = (0, ROWS_A) if half == 0 else (ROWS_A, NLOCP)
            t0 = NC * r0
            nc.gpsimd.collective_compute(
                "AllGather",
                mybir.AluOpType.bypass,
                ins=[bounces[li][r0:r1, :]],
                outs=[tables[li][t0 : t0 + NC * (r1 - r0), :]],
                replica_groups=[list(range(NC))],
            )

        # ---- layer-0 stage A from x ----
        alloc_layer_tiles(0)
        for gidx, (g0, g1) in enumerate(groups):
            for b in range(g0, g1):
                h_chunk = work.tile([128, 128], bf16, tag="xchunk")
                nc.sync.dma_start(
                    out=h_chunk[:],
                    in_=x_p[:].rearrange("p (c f) -> p c f", f=128)[:, b, :],
                )
                emit_stageA_block(0, b, h_chunk[:])
            emit_bounce_piece(0, g0, g1)
            if gidx == HALF_G - 1:
                emit_allgather(0, 0)
        emit_allgather(0, 1)

        acc_t = [None] * NLAYER
        for li, (fi, fo) in enumerate(DIMS):
            last = li == NLAYER - 1
            fop, fot = FOP[li], FOT[li]
            fa = fot if last else fo
            acc = accp.tile([128, NBLK, fa], f32, tag="acc")
            acc_t[li] = acc
            if not last:
                h_t[li] = hpool.tile([128, NBLK, fo], bf16, tag="h", name=f"h{li}")
                alloc_layer_tiles(li + 1)

            tbl_view = tables[li][:].rearrange("(a b) f -> a (b f)", b=G)
            elem = G * fop
            is8 = tdt[li] == f8
            started = [False] * NBLK
            gi = 0
            for ci in range(NCHUNK):
                c0 = ci * CHUNK_COLS
                msg = msgp.tile([128, CHUNK_COLS, G, fop], tdt[li], tag="msg")
                nc.gpsimd.dma_gather(
                    msg[:].rearrange("p s g f -> p s (g f)"),
                    tbl_view,
                    idx_t[:, c0 * 8 : (c0 + CHUNK_COLS) * 8],
                    CHUNK_COLS * 128,
                    nidx_reg,
                    elem,
                    queue_num=ci % 4,
                )
                if is8:
                    msgw = msgp.tile([128, CHUNK_COLS, G, fot], f16, tag="msgw")
                else:
                    msgw = msg
                nc.vector.tensor_tensor(
                    out=msgw[:, :, :, :fot],
                    in0=msg[:, :, :, :fot],
                    in1=wqg_t[:, c0 : c0 + CHUNK_COLS, :].unsqueeze(-1).to_broadcast(
                        [128, CHUNK_COLS, G, fot]
                    ),
                    op=mybir.AluOpType.mult,
                )
                # reduce block pieces inside this chunk
                b_lo = int(np.searchsorted(colbase, c0, side="right")) - 1
                b_hi = int(np.searchsorted(colbase, c0 + CHUNK_COLS, side="left"))
                for b in range(b_lo, min(b_hi, NBLK)):
                    s0 = max(int(colbase[b]), c0) - c0
                    s1 = min(int(colbase[b + 1]), c0 + CHUNK_COLS) - c0
                    if s1 <= s0:
                        continue
                    view = msgw[:, s0:s1, :, :fot].rearrange("p s g f -> p f (s g)")
                    if not started[b]:
                        nc.vector.tensor_reduce(
                            acc[:, b, :fot], view, mybir.AxisListType.X, mybir.AluOpType.add
                        )
                        started[b] = True
                    else:
                        pacc = work.tile([128, fot], f32, tag="pacc")
                        nc.vector.tensor_reduce(
                            pacc[:], view, mybir.AxisListType.X, mybir.AluOpType.add
                        )
                        nc.vector.tensor_tensor(
                            out=acc[:, b, :fot], in0=acc[:, b, :fot], in1=pacc[:],
                            op=mybir.AluOpType.add,
                        )
                # interleaved epilogue + next-layer stage A + bounce for groups
                # whose accumulator columns are now fully reduced
                while not last and gi < len(groups) and ready_ci(groups[gi][1]) <= ci:
                    g0, g1 = groups[gi]
                    nc.vector.tensor_tensor(
                        out=acc[:, g0:g1, :fo],
                        in0=acc[:, g0:g1, :fo],
                        in1=htil2_t[li][:, g0:g1, :],
                        op=mybir.AluOpType.add,
                    )
                    nc.scalar.activation(
                        out=h_t[li][:, g0:g1, :],
                        in_=acc[:, g0:g1, :fo],
                        func=mybir.ActivationFunctionType.Relu,
                    )
                    for b in range(g0, g1):
                        emit_stageA_block(li + 1, b, h_t[li][:, b, :])
                    emit_bounce_piece(li + 1, g0, g1)
                    if gi == HALF_G - 1:
                        emit_allgather(li + 1, 0)
                    gi += 1
            if not last:
                emit_allgather(li + 1, 1)
            else:
                # out = (acc + dinv*htil7) @ W7 + b7 ; htil7 = dinv*h6
                for b in range(NBLK):
                    nc.vector.scalar_tensor_tensor(
                        out=acc[:, b, :],
                        in0=htil_t[li][:, b, :fot],
                        scalar=dinv_t[:, b : b + 1],
                        in1=acc[:, b, :],
                        op0=mybir.AluOpType.mult,
                        op1=mybir.AluOpType.add,
                    )
                nc.vector.tensor_tensor(
                    out=acc[:],
                    in0=acc[:],
                    in1=w7row_t[:].unsqueeze(1).to_broadcast([128, NBLK, fot]),
                    op=mybir.AluOpType.mult,
                )
                out_sb = work.tile([128, NBLK], f32, tag="outsb")
                nc.vector.tensor_reduce(
                    out_sb[:], acc[:], mybir.AxisListType.X, mybir.AluOpType.add
                )
                nc.vector.tensor_tensor(
                    out=out_sb[:],
                    in0=out_sb[:],
                    in1=B_ts[6][:, 0:1].to_broadcast([128, NBLK]),
                    op=mybir.AluOpType.add,
                )
        nc.sync.dma_start(out=out_ext[:], in_=out_sb[:])

    nc.finalize()
    return nc


LAST_EXEC_NS = None
LAST_TRACE = None


def kernel(x, edge_index, edge_weight, W1, b1, W2, b2, W3, b3, W4, b4, W5, b5, W6, b6, W7, b7):
    import os

    from concourse.bass_utils import run_bass_kernel_spmd

    x = np.asarray(x, dtype=np.float32)
    row = np.asarray(edge_index[0], dtype=np.int64)
    col = np.asarray(edge_index[1], dtype=np.int64)
    w = np.asarray(edge_weight, dtype=np.float32)

    # --- host prep with a COMMON slot structure across cores ---
    deg = np.bincount(col, minlength=N_NODES)
    deg_w = np.bincount(col, weights=w.astype(np.float64), minlength=N_NODES) + 1.0
    dinv = (1.0 / np.sqrt(deg_w)).astype(np.float32)
    place_p = np.zeros(N_NODES, np.int64)
    place_c = np.zeros(N_NODES, np.int64)
    for k in range(NC):
        lo, hi = k * NLOC, (k + 1) * NLOC
        order = np.argsort(-deg[lo:hi], kind="stable")
        rank = np.empty(NLOC, np.int64)
        rank[order] = np.arange(NLOC)
        place_p[lo:hi] = rank % 128
        place_c[lo:hi] = rank // 128
    # split table numbering: [all cores' half-A rows][all cores' half-B rows]
    rows_a = HALF_BLK * 128
    core_id = np.arange(N_NODES) // NLOC
    node_row = np.where(
        place_c < HALF_BLK,
        core_id * rows_a + place_c * 128 + place_p,
        NC * rows_a + core_id * (NLOCP - rows_a) + (place_c - HALF_BLK) * 128 + place_p,
    )

    core_of = col // NLOC
    blk_deg_max = np.zeros(NBLK, np.int64)
    per_core = []
    for k in range(NC):
        m = core_of == k
        r_k, c_k, w_k = row[m], col[m], w[m]
        p_d, c_d = place_p[c_k], place_c[c_k]
        bd = np.zeros((128, NBLK), np.int64)
        np.add.at(bd, (p_d, c_d), 1)
        blk_deg_max = np.maximum(blk_deg_max, bd.max(axis=0))
        per_core.append((r_k, w_k, c_k, p_d, c_d))
    D_b = np.maximum(blk_deg_max, 1)
    colbase = np.zeros(NBLK + 1, np.int64)
    colbase[1:] = np.cumsum(D_b)
    NSLOT = int(colbase[-1])
    NSLOT = ((NSLOT + CHUNK_COLS - 1) // CHUNK_COLS) * CHUNK_COLS

    in_maps = []
    import ml_dtypes
    for k in range(NC):
        r_k, w_k, c_k, p_d, c_d = per_core[k]
        key = c_d * 128 + p_d
        order = np.argsort(key, kind="stable")
        r_s, w_s, cd_s, p_s, c_s = r_k[order], w_k[order], c_k[order], p_d[order], c_d[order]
        ks = key[order]
        first = np.zeros(128 * NBLK + 1, np.int64)
        first[1:] = np.cumsum(np.bincount(ks, minlength=128 * NBLK))
        j = np.arange(len(ks), dtype=np.int64) - first[ks]
        slotcol = colbase[c_s] + j

        g_idx = np.zeros((128, NSLOT), np.int16)
        wqg = np.zeros((128, NSLOT, G), np.float16)
        src_row = node_row[r_s]
        g_idx[p_s, slotcol] = (src_row // G).astype(np.int16)
        wqg[p_s, slotcol, src_row % G] = (w_s * dinv[cd_s]).astype(np.float16)

        arr = g_idx.T.reshape(-1)
        wrapped = arr.reshape(-1, 16).T
        idx_w = np.tile(wrapped, (8, 1)).astype(np.int16)

        lo = k * NLOC
        xk = np.zeros((128, NBLK, 128), np.float32)
        xk[place_p[lo : lo + NLOC], place_c[lo : lo + NLOC]] = x[lo : lo + NLOC]
        xk_bf = xk.astype(ml_dtypes.bfloat16)

        dv = np.zeros((128, NBLK), np.float32)
        dv[place_p[lo : lo + NLOC], place_c[lo : lo + NLOC]] = dinv[lo : lo + NLOC]

        in_maps.append(
            {
                "idx_w": idx_w,
                "wqg": wqg.reshape(128, NSLOT * G),
                "x_p": xk_bf.reshape(128, NBLK * 128),
                "dinv_p": dv,
                "dinv2_p": dv * dv,
            }
        )

    Wmats = [np.asarray(Wm, dtype=np.float32) for Wm in (W1, W2, W3, W4, W5, W6, W7)]
    bvecs = [np.tile(np.asarray(b, dtype=np.float32).reshape(1, -1), (128, 1)) for b in (b1, b2, b3, b4, b5, b6, b7)]
    ident = np.eye(128, dtype=np.float32)
    w7r = np.tile(Wmats[6][:, 0].reshape(1, -1), (128, 1)).astype(np.float32)
    for mdl in in_maps:
        for i in range(NLAYER):
            mdl[f"W{i+1}"] = Wmats[i]
            mdl[f"b{i+1}"] = bvecs[i]
        mdl["ident"] = ident
        mdl["w7row"] = w7r

    nc = _build_program(NSLOT, colbase)
    trace = os.environ.get("BASS_GCN_TRACE", "0") == "1"
    kw = {}
    if trace:
        kw = dict(trace=True, tmpdir="/tmp/gcn_trace")
        os.makedirs("/tmp/gcn_trace", exist_ok=True)
    res = run_bass_kernel_spmd(nc, in_maps, list(range(NC)), **kw)
    global LAST_EXEC_NS, LAST_TRACE
    LAST_EXEC_NS = res.exec_time_ns
    LAST_TRACE = res.instructions_and_trace[1] if res.instructions_and_trace else None

    out = np.zeros((N_NODES, 1), np.float32)
    for k in range(NC):
        pm = res.results[k]["out"]  # [128, NBLK]
        lo = k * NLOC
        out[lo : lo + NLOC, 0] = pm[place_p[lo : lo + NLOC], place_c[lo : lo + NLOC]]
    return out
